# revision 12
# baseline (speedup 1.0000x reference)
"""ChildSum TreeGRU on 8 Trainium2 NeuronCores.

Data-parallel over trees (16 trees/core). Feature-major device layout
([256 feat] -> 2x128 partitions, nodes on the free dim). All compute tensors
are float16: fp16 keeps the PE at 1 cycle/row and -- the key win over the
f32 baseline -- every DVE tensor_tensor op qualifies for the 2x_1p perf mode
(2-byte dtype, packed last dim), halving vector-engine time.

Within each tree every level is stored in bit-reversed node order: children
of the parent at stored position j (level l) sit at positions j and j + 2^l
of level l+1, so all child-sum / gate combines are contiguous block ops.
The host permutes the leaf slice of x on the way in and un-permutes the
output (bit reversal is an involution, one index array serves both).

Bulk levels use the all-tensor_tensor combine
    d_k = h_k - hcand ; e_k = z_k * d_k ; h_new = (e_l + e_r) + hcand
(scalar_tensor_tensor has no DVE perf mode). The latency-bound tail levels
(6..0) instead use
    zs = z_l + z_r ; f = z*hc ; g = f_l + f_r   (all before hcand lands)
    t = (zs - 1)*hcand ; h_new = g - t          (2-op tail after hcand)
which shortens the post-hcand critical path.

Schedule: groups of 4 trees run a wavefront over stages (leaf, lv9, lv8,
lv7); level 6 runs as two 8-tree half-stages tucked under the wavefront
tail; levels 5..0 run jointly from a resident buffer (levels 0..7).
Output DMAs issue from the GpSimd queue so they never head-of-line block
the Sync queue that feeds x. Weights/biases are packed into single DMAs.
"""
import sys

for p in ("/opt/trn_rl_repo", "/root/.axon_site/_ro/trn_rl_repo"):
    if p not in sys.path:
        sys.path.insert(0, p)

import numpy as np
import concourse.tile as tile
from concourse import bacc, mybir
from concourse.bass_utils import run_bass_kernel_spmd

f32 = mybir.dt.float32
f16 = mybir.dt.float16
AF = mybir.ActivationFunctionType
ALU = mybir.AluOpType

T, DEPTH, NN, H = 128, 11, 2047, 256
NCORES = 8
TPC = T // NCORES          # 16 trees per core
# progressive group sizes: the last groups are small so the final group's
# serial lv9->lv8->lv7 chain (the endgame critical path) is short
GROUPS = [(0, 4), (4, 4), (8, 4), (12, 2), (14, 2)]
NG = len(GROUPS)
GMAX = 4
NLEAF = 1 << (DEPTH - 1)   # 1024
LEAF0 = NLEAF - 1          # 1023
JN = 255                   # nodes/tree resident in jbuf (levels 0..7)
PS_COLS = 1024             # psum chunk (2 banks) consumed by one ACT
WIDX = {"w": 0, "uz": 1, "ur": 2, "uc": 3}
BIDX = {"bw": 0, "bz": 1, "br": 2, "bc": 3}


def _bitrev_perm(nbits):
    n = 1 << nbits
    p = np.arange(n)
    out = np.zeros(n, dtype=np.int64)
    for b in range(nbits):
        out |= ((p >> b) & 1) << (nbits - 1 - b)
    return out


# device column (within-tree) -> natural heap node id; involution per level
_NAT = np.concatenate(
    [(1 << l) - 1 + _bitrev_perm(l) for l in range(DEPTH)])


def _mm_act_stream(nc, P, pool, tag, lhs, chunks, act_out, func, bias_ap):
    """Matmul+ACT over psum chunks, k-outer across chunk pairs so each
    weight tile loads once per pair (fewer PE weight-switch stalls)."""
    for i in range(0, len(chunks), 2):
        pair = chunks[i:i + 2]
        tiles = []
        for (c0, pn, halves) in pair:
            tiles.append(P[pool].tile([128, pn], f32,
                                      name=f"ps{tag}_{c0}", tag=pool))
        for k in range(2):
            for ps, (c0, pn, halves) in zip(tiles, pair):
                for off, rhs in halves[k]:
                    nc.tensor.matmul(ps[:, off:off + rhs.free_size()],
                                     lhs[k], rhs, start=(k == 0), stop=(k == 1))
        for ps, (c0, pn, halves) in zip(tiles, pair):
            nc.scalar.activation(act_out[:, c0:c0 + pn], ps[:], func, bias=bias_ap)


def _flat_chunks(src, total):
    """[(c0, pn, halves)] over a contiguous per-half source (tile list)."""
    out = []
    for c0 in range(0, total, PS_COLS):
        pn = min(PS_COLS, total - c0)
        halves = [[], []]
        for k in range(2):
            for s0 in range(0, pn, 512):
                n = min(512, pn - s0)
                halves[k].append((s0, src[k][:, c0 + s0:c0 + s0 + n]))
        out.append((c0, pn, halves))
    return out


def _view_chunks(hc3, NT, Lct):
    """[(c0, pn, halves)] over a 3-D [128, NT, Lct] child view (jbuf)."""
    out = []
    tch = max(1, PS_COLS // Lct)
    for t0 in range(0, NT, tch):
        t1 = min(NT, t0 + tch)
        pn = (t1 - t0) * Lct
        halves = [[], []]
        for k in range(2):
            nsub = max(1, pn // 512)
            tsub = max(1, (t1 - t0) // nsub)
            for ts in range(t0, t1, tsub):
                te = min(t1, ts + tsub)
                halves[k].append(((ts - t0) * Lct, hc3[k][:, ts:te, :]))
        out.append((t0 * Lct, pn, halves))
    return out


def _emit_level(nc, P, tag, NT, Lct, hc3, hc_flat, out3, Wt, bias, form, pools):
    """One GRU level for NT trees with Lct children per tree."""
    Lpt = Lct // 2
    Lc = NT * Lct
    Lp = NT * Lpt
    p_hs, p_r, p_hc, p_z = pools

    def lhs_of(nm, m):
        return [Wt[k][:, WIDX[nm] * 256 + m * 128:WIDX[nm] * 256 + (m + 1) * 128]
                for k in range(2)]

    # --- h_sum = lefts + rights (packed block add) ---
    hs = [P[p_hs].tile([128, Lp], f16, name=f"hs{tag}_{m}", tag=f"{p_hs}hs{m}")
          for m in range(2)]
    for m in range(2):
        h3 = hs[m][:].rearrange("p (t n) -> p t n", t=NT)
        nc.vector.tensor_tensor(h3, hc3[m][:, :, 0:Lpt], hc3[m][:, :, Lpt:Lct], ALU.add)

    # --- r = sigmoid(Ur @ h_sum + br) ---
    r = [P[p_r].tile([128, Lp], f16, name=f"r{tag}_{m}", tag=f"{p_r}r{m}")
         for m in range(2)]
    for m in range(2):
        _mm_act_stream(nc, P, "psrc", f"r{tag}{m}", lhs_of("ur", m),
                       _flat_chunks(hs, Lp), r[m][:], AF.Sigmoid, bias["br"][m])

    # --- z = sigmoid(Uz @ hc + bz) over all children ---
    z = [P[p_z].tile([128, Lc], f16, name=f"z{tag}_{m}", tag=f"{p_z}z{m}")
         for m in range(2)]
    zch = (_flat_chunks(hc_flat, Lc) if hc_flat is not None
           else _view_chunks(hc3, NT, Lct))
    for m in range(2):
        _mm_act_stream(nc, P, "psz", f"z{tag}{m}", lhs_of("uz", m),
                       zch, z[m][:], AF.Sigmoid, bias["bz"][m])

    # --- rh = r * h_sum (in place into hs) ---
    for m in range(2):
        nc.vector.tensor_tensor(hs[m][:], r[m][:], hs[m][:], ALU.mult)

    if form == "stt":
        # zs/f/g run as soon as z is ready, overlapping the c-chain
        zs = [P[p_hs].tile([128, Lp], f16, name=f"zs{tag}_{m}", tag=f"{p_hs}zs{m}")
              for m in range(2)]
        g = [P[p_hs].tile([128, Lp], f16, name=f"g{tag}_{m}", tag=f"{p_hs}g{m}")
             for m in range(2)]
        for m in range(2):
            z3 = z[m][:].rearrange("p (t n) -> p t n", t=NT)
            zs3 = zs[m][:].rearrange("p (t n) -> p t n", t=NT)
            nc.vector.tensor_tensor(zs3, z3[:, :, 0:Lpt], z3[:, :, Lpt:Lct], ALU.add)
            # f = z * hc, in place into z
            if hc_flat is not None:
                nc.vector.tensor_tensor(z[m][:], z[m][:], hc_flat[m][:], ALU.mult)
            else:
                nc.vector.tensor_tensor(z3, z3, hc3[m], ALU.mult)
            g3 = g[m][:].rearrange("p (t n) -> p t n", t=NT)
            nc.vector.tensor_tensor(g3, z3[:, :, 0:Lpt], z3[:, :, Lpt:Lct], ALU.add)

    # --- h_cand = tanh(Uc @ rh + bc) ---
    hcand = [P[p_hc].tile([128, Lp], f16, name=f"hcand{tag}_{m}", tag=f"{p_hc}hc{m}")
             for m in range(2)]
    for m in range(2):
        _mm_act_stream(nc, P, "psrc", f"c{tag}{m}", lhs_of("uc", m),
                       _flat_chunks(hs, Lp), hcand[m][:], AF.Tanh, bias["bc"][m])

    if form == "stt":
        # t = (zs - 1) * hcand (in place into zs); h_new = g - t
        for m in range(2):
            nc.vector.scalar_tensor_tensor(zs[m][:], zs[m][:], 1.0, hcand[m][:],
                                           ALU.subtract, ALU.mult)
            zs3 = zs[m][:].rearrange("p (t n) -> p t n", t=NT)
            g3 = g[m][:].rearrange("p (t n) -> p t n", t=NT)
            nc.vector.tensor_tensor(out3[m], g3, zs3, ALU.subtract)
    else:
        # d = hc - hcand ; e = z*d ; h_new = (e_l + e_r) + hcand
        for m in range(2):
            hcand3 = hcand[m][:].rearrange("p (t n) -> p t n", t=NT)
            d = P[p_z].tile([128, Lc], f16, name=f"d{tag}_{m}", tag=f"{p_z}z{m}")
            d3 = d[:].rearrange("p (t n) -> p t n", t=NT)
            nc.vector.tensor_tensor(d3[:, :, 0:Lpt], hc3[m][:, :, 0:Lpt], hcand3, ALU.subtract)
            nc.vector.tensor_tensor(d3[:, :, Lpt:Lct], hc3[m][:, :, Lpt:Lct], hcand3, ALU.subtract)
            nc.vector.tensor_tensor(d[:], z[m][:], d[:], ALU.mult)
            s = P[p_hs].tile([128, Lp], f16, name=f"s{tag}_{m}", tag=f"{p_hs}hs{m}")
            s3 = s[:].rearrange("p (t n) -> p t n", t=NT)
            nc.vector.tensor_tensor(s3, d3[:, :, 0:Lpt], d3[:, :, Lpt:Lct], ALU.add)
            nc.vector.tensor_tensor(out3[m], s3, hcand3, ALU.add)


def _build():
    nc = bacc.Bacc("TRN2", debug=False)

    xT_d = nc.dram_tensor("xT", [H, TPC * NLEAF], f16, kind="ExternalInput")
    wpk_d = nc.dram_tensor("wpk", [H, 4 * H], f16, kind="ExternalInput")
    bpk_d = nc.dram_tensor("bpk", [H, 4], f32, kind="ExternalInput")
    hout_d = nc.dram_tensor("h_out", [H, TPC, NN], f16, kind="ExternalOutput")

    with tile.TileContext(nc) as tc:
        from contextlib import ExitStack
        with ExitStack() as ctx:
            P = {}
            P["const"] = ctx.enter_context(tc.tile_pool(name="const", bufs=1))
            P["xg"] = ctx.enter_context(tc.tile_pool(name="xg", bufs=2))
            P["h10"] = ctx.enter_context(tc.tile_pool(name="h10", bufs=2))
            P["hl"] = ctx.enter_context(tc.tile_pool(name="hl", bufs=2))
            P["jbuf"] = ctx.enter_context(tc.tile_pool(name="jbuf", bufs=1))
            P["z"] = ctx.enter_context(tc.tile_pool(name="z", bufs=3))
            P["hs"] = ctx.enter_context(tc.tile_pool(name="hs", bufs=2))
            P["r"] = ctx.enter_context(tc.tile_pool(name="r", bufs=1))
            P["hc"] = ctx.enter_context(tc.tile_pool(name="hc", bufs=1))
            P["jz"] = ctx.enter_context(tc.tile_pool(name="jz", bufs=1))
            P["jsm"] = ctx.enter_context(tc.tile_pool(name="jsm", bufs=1))
            P["psz"] = ctx.enter_context(tc.tile_pool(name="psz", bufs=2, space="PSUM"))
            P["psrc"] = ctx.enter_context(tc.tile_pool(name="psrc", bufs=2, space="PSUM"))

            cp = P["const"]
            # packed weights: one [128, 1024] DMA per contraction half
            Wt = [cp.tile([128, 4 * H], f16, name=f"wpk{k}") for k in range(2)]
            for k in range(2):
                nc.sync.dma_start(Wt[k][:], wpk_d.ap()[k * 128:(k + 1) * 128, :])
            bt = [cp.tile([128, 4], f32, name=f"bpk{m}") for m in range(2)]
            for m in range(2):
                nc.sync.dma_start(bt[m][:], bpk_d.ap()[m * 128:(m + 1) * 128, :])
            bias = {nm: [bt[m][:, j:j + 1] for m in range(2)]
                    for nm, j in BIDX.items()}

            grp_pools = ("hs", "r", "hc", "z")
            jnt_pools = ("jsm", "jsm", "jsm", "jz")

            # joint buffer: bitrev heap levels 0..7 for all 16 trees, per half
            jbuf = [P["jbuf"].tile([128, TPC * JN], f16, name=f"jbuf{m}") for m in range(2)]
            jv = [jbuf[m][:].rearrange("p (t n) -> p t n", t=TPC) for m in range(2)]

            xgs = {}

            def emit_xdma(g):
                t0, NT = GROUPS[g]
                xg = [P["xg"].tile([128, NT * NLEAF], f16, name=f"xg{g}_{k}", tag=f"xg{k}")
                      for k in range(2)]
                for piece in range(0, NT * NLEAF, 2048):
                    pend = min(piece + 2048, NT * NLEAF)
                    for k in range(2):
                        nc.sync.dma_start(
                            xg[k][:, piece:pend],
                            xT_d.ap()[k * 128:(k + 1) * 128,
                                      t0 * NLEAF + piece:t0 * NLEAF + pend])
                xgs[g] = xg

            def emit_leaf(g):
                t0, NT = GROUPS[g]
                xg = xgs[g]
                h10 = [P["h10"].tile([128, NT * NLEAF], f16, name=f"h10g{g}_{m}", tag=f"h10{m}")
                       for m in range(2)]
                for m in range(2):
                    lhs = [Wt[k][:, WIDX["w"] * 256 + m * 128:WIDX["w"] * 256 + (m + 1) * 128]
                           for k in range(2)]
                    _mm_act_stream(nc, P, "psz", f"xg{g}{m}", lhs,
                                   _flat_chunks(xg, NT * NLEAF), h10[m][:],
                                   AF.Tanh, bias["bw"][m])
                    nc.gpsimd.dma_start(
                        hout_d.ap()[m * 128:(m + 1) * 128, t0:t0 + NT,
                                    LEAF0:LEAF0 + NLEAF],
                        h10[m][:].rearrange("p (t n) -> p t n", t=NT))
                return h10

            def emit_lvl(g, lv, hchild):
                t0, NT = GROUPS[g]
                Lct = 2 ** (lv + 1)
                Lpt = 2 ** lv
                hc3 = [hchild[m][:].rearrange("p (t n) -> p t n", t=NT) for m in range(2)]
                hc_flat = [hchild[m][:] for m in range(2)]
                if lv == 7:
                    out3 = [jv[m][:, t0:t0 + NT, Lpt - 1:2 * Lpt - 1]
                            for m in range(2)]
                    hnew = None
                else:
                    hnew = [P["hl"].tile([128, NT * Lpt], f16,
                                         name=f"h{lv}g{g}_{m}", tag=f"h{lv}_{m}")
                            for m in range(2)]
                    out3 = [hnew[m][:].rearrange("p (t n) -> p t n", t=NT)
                            for m in range(2)]
                _emit_level(nc, P, f"g{g}l{lv}", NT, Lct, hc3, hc_flat, out3,
                            Wt, bias, "d", grp_pools)
                for m in range(2):
                    src = (jv[m][:, t0:t0 + NT, Lpt - 1:2 * Lpt - 1]
                           if lv == 7 else
                           hnew[m][:].rearrange("p (t n) -> p t n", t=NT))
                    nc.gpsimd.dma_start(
                        hout_d.ap()[m * 128:(m + 1) * 128, t0:t0 + NT,
                                    Lpt - 1:2 * Lpt - 1],
                        src)
                return hnew

            def emit_joint(lv, t0, t1):
                # parents at level lv for trees [t0, t1), children from jbuf
                Lct = 2 ** (lv + 1)
                Lpt = 2 ** lv
                NT = t1 - t0
                hc3 = [jv[m][:, t0:t1, Lct - 1:2 * Lct - 1] for m in range(2)]
                out3 = [jv[m][:, t0:t1, Lpt - 1:2 * Lpt - 1] for m in range(2)]
                _emit_level(nc, P, f"j{lv}t{t0}", NT, Lct, hc3, None, out3,
                            Wt, bias, "stt", jnt_pools)
                for m in range(2):
                    nc.gpsimd.dma_start(
                        hout_d.ap()[m * 128:(m + 1) * 128, t0:t1, Lpt - 1:2 * Lpt - 1],
                        jv[m][:, t0:t1, Lpt - 1:2 * Lpt - 1])

            # wavefront: stage s of group g at tick t = g + s (0=leaf, 1..3 =
            # levels 9..7); x for group t+1 prefetches one tick ahead; level 6
            # runs in pieces as soon as their trees' lv7 lands, tucked under
            # the wavefront tail
            lv6_at = {5: (0, 8), 6: (8, 12), 7: (12, 14)}
            emit_xdma(0)
            gstate = {}
            for t in range(NG + 3):
                if t + 1 < NG:
                    emit_xdma(t + 1)
                # past the last leaf, the latest group's serial chain is the
                # critical path: put its ops early in the in-order queues
                order = reversed(range(NG)) if t >= NG else range(NG)
                for g in order:
                    s = t - g
                    if s < 0 or s > 3:
                        continue
                    if s == 0:
                        gstate[g] = emit_leaf(g)
                    else:
                        gstate[g] = emit_lvl(g, 10 - s, gstate[g])
                if t in lv6_at:
                    emit_joint(6, *lv6_at[t])
            emit_joint(6, 14, 16)

            for lv in range(5, -1, -1):
                emit_joint(lv, 0, TPC)

    nc.compile()
    return nc


_NC = None


def _get_nc():
    global _NC
    if _NC is None:
        _NC = _build()
    return _NC


def make_in_maps(inputs):
    x = np.asarray(inputs["x"], np.float32)
    W = np.asarray(inputs["W"], np.float32)
    bW = np.asarray(inputs["bW"], np.float32).reshape(H, 1)
    Ur = np.asarray(inputs["Ur"], np.float32)
    br = np.asarray(inputs["br"], np.float32).reshape(H, 1)
    Uc = np.asarray(inputs["Uc"], np.float32)
    bc = np.asarray(inputs["bc"], np.float32).reshape(H, 1)
    Uz = np.asarray(inputs["Uz"], np.float32)
    bz = np.asarray(inputs["bz"], np.float32).reshape(H, 1)
    wpk = np.concatenate(
        [np.ascontiguousarray(M.T) for M in (W, Uz, Ur, Uc)],
        axis=1).astype(np.float16)                    # [256, 1024]
    bpk = np.concatenate([bW, bz, br, bc], axis=1)    # [256, 4] f32
    shared = {"wpk": wpk, "bpk": np.ascontiguousarray(bpk)}
    leaf_nat = _NAT[LEAF0:]          # absolute heap ids of bitrev'd leaves
    in_maps = []
    for c in range(NCORES):
        xs = x[c * TPC:(c + 1) * TPC, leaf_nat, :]           # [16, 1024, 256]
        xTc = np.ascontiguousarray(xs.transpose(2, 0, 1)).reshape(H, TPC * NLEAF)
        in_maps.append({"xT": xTc.astype(np.float16), **shared})
    return in_maps


def assemble_out(core_outs):
    out = np.empty((T, NN, H), np.float32)
    for c in range(NCORES):
        # [256, 16, 2047] f16 (bitrev levels) -> [16, 2047, 256] f32 natural
        dev = core_outs[c].transpose(1, 2, 0).astype(np.float32)
        out[c * TPC:(c + 1) * TPC] = dev[:, _NAT, :]
    return out


def kernel(**inputs):
    nc = _get_nc()
    in_maps = make_in_maps(inputs)
    res = run_bass_kernel_spmd(nc, in_maps, list(range(NCORES)))
    return assemble_out([r["h_out"] for r in res.results])
